# revision 26
# baseline (speedup 1.0000x reference)
"""CrossNetwork kernel for TRN2, 8-core data-parallel, bf16 I/O.

Reference computation (per layer i in 0..3):
    s_i = <x_i, w_i>            (per-sample dot, feature dim 1024)
    x_{i+1} = x0 * s_i + b_i + x_i

Algebraic collapse: x_i = a_i * x0 + d_i with
    d_{i+1} = d_i + b_i                  (sample-independent vectors)
    a_{i+1} = a_i * (1 + u_i) + e_i      (per-sample scalars)
where u_i = <x0, w_i> and e_i = <d_i, w_i>.  Output = a_4 * x0 + d_4;
the d_4 term is ~1e-7 of the output scale and is dropped.

Precision budget: the rel-err gate is 2e-2 against absmax(expected).
bf16 x (input + final multiply + output) and bf16 dot operands give
~5.7e-3 end-to-end (measured vs the fp32 reference in numpy) -- 3.5x
margin.  d_i / e_i are computed host-side in float64.

Device plan (per core, B_LOCAL=2048 rows):
  - HBM traffic halves vs fp32: 4.2 MB in + 4.2 MB out (bf16).
  - All weight-derived constants (wt_pack, e_wide, identity) are
    host-packed into exactly the SBUF layouts needed -- no on-device
    weight prep, no GPSIMD broadcasts.
  - x streams in as 8 chunks x 2 tiles on the sync queue (one ~600ns
    trigger each); weight constants load via the scalar queue so they
    don't delay the x stream.
  - Per tile [128 x 1024]: PE transposes 8 bf16 blocks -> PSUM(bf16),
    DVE/ACT copies to SBUF, PE runs 8 accumulating bf16 matmuls
    against wt_pack -> u[128, 4] (fp32 PSUM), ACT copies u out.
  - Per group of 4 tiles: DVE recurrence a = (u_i+1)*a + e_i.
  - Finals out = a * x (bf16) alternate ACT/DVE; output DMA chunks of
    2 tiles trigger on the sync queue behind the input triggers.
  - Group tails are emitted one group behind the dots (tail_cb) so the
    ACT queue never heads-of-line-blocks the next group's u copies.
"""

import numpy as np
import ml_dtypes

N_FEAT = 1024
N_LAYER = 4
B_FULL = 16384
N_CORES = 8
B_LOCAL = B_FULL // N_CORES      # 2048
P = 128                          # SBUF partitions
N_TILES = B_LOCAL // P           # 16
N_BLK = N_FEAT // P              # 8 feature blocks per tile
N_GROUPS = 4
GROUP = N_TILES // N_GROUPS      # 4
# input DMA chunking in tiles: small first chunks so tile 0's semaphore
# fires early, 2-tile chunks after
IN_CHUNKS = [1, 1, 2, 2, 2, 2, 2, 2, 2]
# output DMA chunking: 2 tiles = 4KB per-partition descriptors; the last
# group goes out tile-by-tile -- at the tail the stream is empty, so
# trigger latency beats descriptor efficiency
OUT_CHUNKS = [2, 2, 2, 2, 2, 2, 1, 1, 1, 1]
N_WARMUP_MM = 8                  # burst sized to end right as tile 0's data
                                 # lands (~11us): 8 cold MMs cover the 3.4us
                                 # HAM window

BF16 = ml_dtypes.bfloat16

# final-multiply engine per tile: DVE tensor_scalar is ~480ns, ACT
# activation ~1.23us; 2 finals per group go to ACT to level the queues,
# except the last group where serial DVE (4x0.48) beats ACT stragglers
FIN_ACT = [t % 4 in (0, 2) and t < 12 for t in range(N_TILES)]

_CACHE = {}


def _build_nc():
    import concourse.bass as bass
    import concourse.tile as tile
    from concourse import bacc, mybir

    fp32 = mybir.dt.float32
    bf16 = mybir.dt.bfloat16
    Alu = mybir.AluOpType
    Act = mybir.ActivationFunctionType

    nc = bacc.Bacc(target_bir_lowering=False)

    # x and out live in DRAM partition-major ([p, t, f]) so every DMA
    # descriptor is >=4KB contiguous per partition -- at bf16 the natural
    # row-major layout gives 2KB descriptors and DMA drops to ~250 B/ns
    # from packet overhead
    xb_d = nc.dram_tensor("xb", [P, N_TILES * N_FEAT], bf16, kind="ExternalInput")
    wt_d = nc.dram_tensor("wt_pack", [P, N_BLK * N_LAYER], bf16, kind="ExternalInput")
    ew_d = nc.dram_tensor("e_wide", [P, N_LAYER * GROUP], fp32, kind="ExternalInput")
    id_d = nc.dram_tensor("ident", [P, P], bf16, kind="ExternalInput")
    o_d = nc.dram_tensor("out", [P, N_TILES * N_FEAT], bf16, kind="ExternalOutput")

    with tile.TileContext(nc) as tc:
        with (
            tc.tile_pool(name="const", bufs=1) as cpool,
            tc.tile_pool(name="xtbuf", bufs=3) as xtpool,
            tc.tile_pool(name="psT", bufs=3, space="PSUM") as psT,
            tc.tile_pool(name="psU", bufs=2, space="PSUM") as psU,
        ):
            ident = cpool.tile([P, P], bf16)
            wt = cpool.tile([P, N_BLK * N_LAYER], bf16)
            ew = cpool.tile([P, N_LAYER, GROUP], fp32)
            xb = cpool.tile([P, N_TILES, N_FEAT], bf16)
            ob = cpool.tile([P, N_TILES, N_FEAT], bf16)

            def in_chunk(t0, ntile):
                nc.sync.dma_start(
                    xb[:, t0:t0 + ntile, :],
                    xb_d[:, t0 * N_FEAT:(t0 + ntile) * N_FEAT].rearrange(
                        "p (t f) -> p t f", f=N_FEAT),
                )

            # x chunks own the sync queue end to end (triggers cost ~0.7us
            # each); the tiny weight constants ride the scalar HWDGE ring
            # and land well before their first consumers
            nc.scalar.dma_start(ident[:], id_d[:])
            nc.scalar.dma_start(wt[:], wt_d[:])
            nc.scalar.dma_start(
                ew[:], ew_d[:].rearrange("p (i j) -> p i j", i=N_LAYER))
            t0 = 0
            for ntile in IN_CHUNKS:
                in_chunk(t0, ntile)
                t0 += ntile

            # HAM warmup: junk matmuls on an uninitialized tile so the PE
            # clock is at 2.4 GHz (K=8/8) before the first real transpose
            # lands.  Transpose-mode work does not count as PE-busy for the
            # HAM monitor, so without this the stream runs at 1.2 GHz.  The
            # flip happens ~3.4us after sustained PE activity starts, so the
            # burst length is tuned to end right at the flip.
            junk = cpool.tile([P, 4 * P], bf16)
            nc.gpsimd.memset(junk[:], 0.0)
            junk_ps = psT.tile([P, 4 * P], fp32)
            for _ in range(N_WARMUP_MM):
                nc.tensor.matmul(junk_ps[:], junk[:, :P], junk[:], start=True,
                                 stop=True)

            u_gs = [cpool.tile([P, GROUP, N_LAYER], fp32, name=f"u_g{g}")
                    for g in range(N_GROUPS)]
            a_gs = [cpool.tile([P, GROUP], fp32, name=f"a_g{g}")
                    for g in range(N_GROUPS)]
            a2_gs = [cpool.tile([P, GROUP], fp32, name=f"a2_g{g}")
                     for g in range(N_GROUPS)]

            xt_sbs = [None] * N_TILES

            def emit_transposes(t):
                xt_ps = psT.tile([P, N_FEAT], bf16)
                for f in range(N_BLK):
                    nc.tensor.matmul(
                        xt_ps[:, f * P:(f + 1) * P],
                        xb[:, t, f * P:(f + 1) * P],
                        ident[:],
                        is_transpose=True,
                    )
                xt_sb = xtpool.tile([P, N_FEAT], bf16)
                xt_sbs[t] = xt_sb
                # PSUM eviction split across both streaming engines: DVE
                # half ~390ns, ACT half ~720ns, in parallel
                half = N_FEAT // 2
                nc.vector.tensor_copy(xt_sb[:, :half], xt_ps[:, :half])
                nc.scalar.copy(xt_sb[:, half:], xt_ps[:, half:])

            def emit_dots(t):
                g, j = divmod(t, GROUP)
                xt_sb = xt_sbs[t]
                u_ps = psU.tile([P, N_LAYER], fp32)
                for f in range(N_BLK):
                    nc.tensor.matmul(
                        u_ps[:],
                        xt_sb[:, f * P:(f + 1) * P],
                        wt[:, f * N_LAYER:(f + 1) * N_LAYER],
                        start=(f == 0),
                        stop=(f == N_BLK - 1),
                    )
                nc.scalar.copy(u_gs[g][:, j, :], u_ps[:])

            def emit_rec(g):
                u_g, a_g, a2 = u_gs[g][:], a_gs[g][:], a2_gs[g][:]
                nc.vector.tensor_scalar(a_g, u_g[:, :, 0], 1.0, None, Alu.add)
                for i in range(1, N_LAYER):
                    nc.vector.scalar_tensor_tensor(
                        a2, u_g[:, :, i], 1.0, a_g, Alu.add, Alu.mult)
                    nc.vector.tensor_tensor(a_g, a2, ew[:, i, :], Alu.add)

            out_chunk_end = []
            t_acc = 0
            for ntile in OUT_CHUNKS:
                t_acc += ntile
                out_chunk_end.append(t_acc)

            def emit_final(t):
                g, j = divmod(t, GROUP)
                if FIN_ACT[t]:
                    nc.scalar.activation(
                        ob[:, t, :], xb[:, t, :], Act.Copy,
                        scale=a_gs[g][:, j:j + 1])
                else:
                    nc.vector.tensor_scalar(
                        ob[:, t, :], xb[:, t, :], a_gs[g][:, j:j + 1],
                        None, Alu.mult)
                if t + 1 in out_chunk_end:
                    c = out_chunk_end.index(t + 1)
                    lo = out_chunk_end[c - 1] if c else 0
                    nc.sync.dma_start(
                        o_d[:, lo * N_FEAT:(t + 1) * N_FEAT].rearrange(
                            "p (t f) -> p t f", f=N_FEAT),
                        ob[:, lo:t + 1, :],
                    )

            def make_tail_cb(g_prev):
                emitted = []

                def cb():
                    j = len(emitted)
                    if j < GROUP:
                        emitted.append(j)
                        emit_final(g_prev * GROUP + j)

                def flush():
                    while len(emitted) < GROUP:
                        cb()
                return cb, flush

            # software-pipelined emission: the PE queue at step t holds
            # [transposes(t), dots(t-1)], so while dots(t-1) waits for the
            # xt copy of tile t-1 (on DVE/ACT), the PE is already streaming
            # tile t's transposes -- the PE never head-of-line blocks on a
            # cross-engine copy.  Finals for group g-1 are interleaved one
            # group behind (tail_cb), and the recurrence for group g is
            # emitted as soon as its last u copy is queued.
            tail_cb = None
            flush = None
            for t in range(N_TILES + 1):
                if t < N_TILES:
                    emit_transposes(t)
                if t >= 1:
                    emit_dots(t - 1)
                    if tail_cb is not None:
                        tail_cb()
                    g, j = divmod(t - 1, GROUP)
                    if j == GROUP - 1:
                        if flush is not None:
                            flush()
                        emit_rec(g)
                        tail_cb, flush = make_tail_cb(g)
            flush()

    nc.compile()
    return nc


def _host_prep(weight_w, weight_b):
    """Host-side constants: wt_pack [128, 32] bf16, e_wide [128, 16] f32,
    ident [128, 128] bf16."""
    w64 = weight_w.astype(np.float64)
    b64 = weight_b.astype(np.float64)
    d = np.zeros((N_LAYER + 1, N_FEAT), dtype=np.float64)
    for i in range(N_LAYER):
        d[i + 1] = d[i] + b64[i]
    e = np.array([np.dot(d[i], w64[i]) for i in range(N_LAYER)],
                 dtype=np.float64)

    wbf = weight_w.astype(BF16)                       # [4, 1024]
    wt_pack = np.ascontiguousarray(
        wbf.T.reshape(N_BLK, P, N_LAYER).transpose(1, 0, 2).reshape(
            P, N_BLK * N_LAYER))                      # [p, f*4+i] = w[i, 128f+p]
    e_wide = np.ascontiguousarray(
        np.broadcast_to(
            np.repeat(e.astype(np.float32), GROUP)[None, :],
            (P, N_LAYER * GROUP)))
    ident = np.eye(P, dtype=BF16)
    return wt_pack, e_wide, ident


def _get_nc():
    if "nc" not in _CACHE:
        _CACHE["nc"] = _build_nc()
    return _CACHE["nc"]


def run(x, weight_w, weight_b, trace=False):
    """Run on 8 cores; returns (out_full, BassKernelResults)."""
    from concourse.bass_utils import run_bass_kernel_spmd

    x = np.ascontiguousarray(np.asarray(x, dtype=np.float32))
    weight_w = np.asarray(weight_w, dtype=np.float32)
    weight_b = np.asarray(weight_b, dtype=np.float32)
    assert x.shape == (B_FULL, N_FEAT)

    # partition-major device layout: [core][p, t, f] so DMA descriptors are
    # >=4KB contiguous per partition
    xb = np.ascontiguousarray(
        x.astype(BF16).reshape(N_CORES, N_TILES, P, N_FEAT)
        .transpose(0, 2, 1, 3).reshape(N_CORES, P, N_TILES * N_FEAT))
    wt_pack, e_wide, ident = _host_prep(weight_w, weight_b)

    nc = _get_nc()
    in_maps = [
        {
            "xb": xb[c],
            "wt_pack": wt_pack,
            "e_wide": e_wide,
            "ident": ident,
        }
        for c in range(N_CORES)
    ]
    res = run_bass_kernel_spmd(nc, in_maps, list(range(N_CORES)), trace=trace)
    out = np.concatenate(
        [res.results[c]["out"].reshape(P, N_TILES, N_FEAT).transpose(1, 0, 2)
         .reshape(B_LOCAL, N_FEAT).astype(np.float32)
         for c in range(N_CORES)],
        axis=0)
    return out, res


def kernel(x, weight_w, weight_b):
    out, _ = run(x, weight_w, weight_b, trace=False)
    return out


# revision 27
# speedup vs baseline: 1.0762x; 1.0762x over previous
"""CrossNetwork kernel for TRN2, 8-core data-parallel, bf16 I/O.

Reference computation (per layer i in 0..3):
    s_i = <x_i, w_i>            (per-sample dot, feature dim 1024)
    x_{i+1} = x0 * s_i + b_i + x_i

Algebraic collapse: x_i = a_i * x0 + d_i with
    d_{i+1} = d_i + b_i                  (sample-independent vectors)
    a_{i+1} = a_i * (1 + u_i) + e_i      (per-sample scalars)
where u_i = <x0, w_i> and e_i = <d_i, w_i>.  Output = a_4 * x0 + d_4;
the d_4 term is ~1e-7 of the output scale and is dropped.

Precision budget: the rel-err gate is 2e-2 against absmax(expected).
bf16 x (input + final multiply + output) and bf16 dot operands give
~5.7e-3 end-to-end (measured vs the fp32 reference in numpy) -- 3.5x
margin.  d_i / e_i are computed host-side in float64.

Device plan (per core, B_LOCAL=2048 rows):
  - HBM traffic halves vs fp32: 4.2 MB in + 4.2 MB out (bf16).
  - All weight-derived constants (wt_pack, e_wide, identity) are
    host-packed into exactly the SBUF layouts needed -- no on-device
    weight prep, no GPSIMD broadcasts.
  - x streams in as 8 chunks x 2 tiles on the sync queue (one ~600ns
    trigger each); weight constants load via the scalar queue so they
    don't delay the x stream.
  - Per tile [128 x 1024]: PE transposes 8 bf16 blocks -> PSUM(bf16),
    DVE/ACT copies to SBUF, PE runs 8 accumulating bf16 matmuls
    against wt_pack -> u[128, 4] (fp32 PSUM), ACT copies u out.
  - Per group of 4 tiles: DVE recurrence a = (u_i+1)*a + e_i.
  - Finals out = a * x (bf16) alternate ACT/DVE; output DMA chunks of
    2 tiles trigger on the sync queue behind the input triggers.
  - Group tails are emitted one group behind the dots (tail_cb) so the
    ACT queue never heads-of-line-blocks the next group's u copies.
"""

import numpy as np
import ml_dtypes

N_FEAT = 1024
N_LAYER = 4
B_FULL = 16384
N_CORES = 8
B_LOCAL = B_FULL // N_CORES      # 2048
P = 128                          # SBUF partitions
N_TILES = B_LOCAL // P           # 16
N_BLK = N_FEAT // P              # 8 feature blocks per tile
N_GROUPS = 4
GROUP = N_TILES // N_GROUPS      # 4
# input DMA chunking in tiles: small first chunks so tile 0's semaphore
# fires early, 2-tile chunks after
IN_CHUNKS = [1, 1, 2, 2, 2, 2, 2, 2, 2]
# output DMA chunking: 2 tiles = 4KB per-partition descriptors; the last
# group goes out tile-by-tile -- at the tail the stream is empty, so
# trigger latency beats descriptor efficiency
OUT_CHUNKS = [2, 2, 2, 2, 2, 2, 1, 1, 1, 1]
N_WARMUP_MM = 8                  # burst sized to end right as tile 0's data
                                 # lands (~11us): 8 cold MMs cover the 3.4us
                                 # HAM window

BF16 = ml_dtypes.bfloat16

# final-multiply engine per tile: DVE tensor_scalar is ~480ns, ACT
# activation ~1.23us; 2 finals per group go to ACT to level the queues,
# except the last group where serial DVE (4x0.48) beats ACT stragglers
FIN_ACT = [t % 4 in (0, 2) and t < 12 for t in range(N_TILES)]

_CACHE = {}


def _build_nc():
    import concourse.bass as bass
    import concourse.tile as tile
    from concourse import bacc, mybir

    fp32 = mybir.dt.float32
    bf16 = mybir.dt.bfloat16
    Alu = mybir.AluOpType
    Act = mybir.ActivationFunctionType

    nc = bacc.Bacc(target_bir_lowering=False)

    # x and out live in DRAM partition-major ([p, t, f]) so every DMA
    # descriptor is >=4KB contiguous per partition -- at bf16 the natural
    # row-major layout gives 2KB descriptors and DMA drops to ~250 B/ns
    # from packet overhead
    xb_d = nc.dram_tensor("xb", [P, N_TILES * N_FEAT], bf16, kind="ExternalInput")
    wt_d = nc.dram_tensor("wt_pack", [P, N_BLK * N_LAYER], bf16, kind="ExternalInput")
    ew_d = nc.dram_tensor("e_wide", [P, N_LAYER * GROUP], fp32, kind="ExternalInput")
    id_d = nc.dram_tensor("ident", [P, P], bf16, kind="ExternalInput")
    o_d = nc.dram_tensor("out", [P, N_TILES * N_FEAT], bf16, kind="ExternalOutput")

    with tile.TileContext(nc) as tc:
        with (
            tc.tile_pool(name="const", bufs=1) as cpool,
            tc.tile_pool(name="xtbuf", bufs=3) as xtpool,
            tc.tile_pool(name="psT", bufs=3, space="PSUM") as psT,
            tc.tile_pool(name="psU", bufs=2, space="PSUM") as psU,
        ):
            ident = cpool.tile([P, P], bf16)
            wt = cpool.tile([P, N_BLK * N_LAYER], bf16)
            ew = cpool.tile([P, N_LAYER, GROUP], fp32)
            xb = cpool.tile([P, N_TILES, N_FEAT], bf16)
            ob = cpool.tile([P, N_TILES, N_FEAT], bf16)

            def in_chunk(t0, ntile):
                nc.sync.dma_start(
                    xb[:, t0:t0 + ntile, :],
                    xb_d[:, t0 * N_FEAT:(t0 + ntile) * N_FEAT].rearrange(
                        "p (t f) -> p t f", f=N_FEAT),
                )

            # x chunks own the sync queue end to end (triggers cost ~0.7us
            # each); the tiny weight constants ride the scalar HWDGE ring
            # and land well before their first consumers
            nc.scalar.dma_start(ident[:], id_d[:])
            nc.scalar.dma_start(wt[:], wt_d[:])
            nc.scalar.dma_start(
                ew[:], ew_d[:].rearrange("p (i j) -> p i j", i=N_LAYER))
            t0 = 0
            for ntile in IN_CHUNKS:
                in_chunk(t0, ntile)
                t0 += ntile

            # HAM warmup: junk matmuls on an uninitialized tile so the PE
            # clock is at 2.4 GHz (K=8/8) before the first real transpose
            # lands.  Transpose-mode work does not count as PE-busy for the
            # HAM monitor, so without this the stream runs at 1.2 GHz.  The
            # flip happens ~3.4us after sustained PE activity starts, so the
            # burst length is tuned to end right at the flip.
            junk = cpool.tile([P, 4 * P], bf16)
            nc.gpsimd.memset(junk[:], 0.0)
            junk_ps = psT.tile([P, 4 * P], fp32)
            for _ in range(N_WARMUP_MM):
                nc.tensor.matmul(junk_ps[:], junk[:, :P], junk[:], start=True,
                                 stop=True)

            u_gs = [cpool.tile([P, GROUP, N_LAYER], fp32, name=f"u_g{g}")
                    for g in range(N_GROUPS)]
            a_gs = [cpool.tile([P, GROUP], fp32, name=f"a_g{g}")
                    for g in range(N_GROUPS)]
            a2_gs = [cpool.tile([P, GROUP], fp32, name=f"a2_g{g}")
                     for g in range(N_GROUPS)]

            xt_sbs = [None] * N_TILES

            def emit_transposes(t):
                xt_ps = psT.tile([P, N_FEAT], bf16)
                for f in range(N_BLK):
                    nc.tensor.matmul(
                        xt_ps[:, f * P:(f + 1) * P],
                        xb[:, t, f * P:(f + 1) * P],
                        ident[:],
                        is_transpose=True,
                    )
                xt_sb = xtpool.tile([P, N_FEAT], bf16)
                xt_sbs[t] = xt_sb
                nc.vector.tensor_copy(xt_sb[:], xt_ps[:])

            def emit_dots(t):
                g, j = divmod(t, GROUP)
                xt_sb = xt_sbs[t]
                u_ps = psU.tile([P, N_LAYER], fp32)
                for f in range(N_BLK):
                    nc.tensor.matmul(
                        u_ps[:],
                        xt_sb[:, f * P:(f + 1) * P],
                        wt[:, f * N_LAYER:(f + 1) * N_LAYER],
                        start=(f == 0),
                        stop=(f == N_BLK - 1),
                    )
                nc.scalar.copy(u_gs[g][:, j, :], u_ps[:])

            def emit_rec(g):
                u_g, a_g, a2 = u_gs[g][:], a_gs[g][:], a2_gs[g][:]
                nc.vector.tensor_scalar(a_g, u_g[:, :, 0], 1.0, None, Alu.add)
                for i in range(1, N_LAYER):
                    nc.vector.scalar_tensor_tensor(
                        a2, u_g[:, :, i], 1.0, a_g, Alu.add, Alu.mult)
                    nc.vector.tensor_tensor(a_g, a2, ew[:, i, :], Alu.add)

            out_chunk_end = []
            t_acc = 0
            for ntile in OUT_CHUNKS:
                t_acc += ntile
                out_chunk_end.append(t_acc)

            def emit_final(t):
                g, j = divmod(t, GROUP)
                if FIN_ACT[t]:
                    nc.scalar.activation(
                        ob[:, t, :], xb[:, t, :], Act.Copy,
                        scale=a_gs[g][:, j:j + 1])
                else:
                    nc.vector.tensor_scalar(
                        ob[:, t, :], xb[:, t, :], a_gs[g][:, j:j + 1],
                        None, Alu.mult)
                if t + 1 in out_chunk_end:
                    c = out_chunk_end.index(t + 1)
                    lo = out_chunk_end[c - 1] if c else 0
                    nc.sync.dma_start(
                        o_d[:, lo * N_FEAT:(t + 1) * N_FEAT].rearrange(
                            "p (t f) -> p t f", f=N_FEAT),
                        ob[:, lo:t + 1, :],
                    )

            def make_tail_cb(g_prev):
                emitted = []

                def cb():
                    j = len(emitted)
                    if j < GROUP:
                        emitted.append(j)
                        emit_final(g_prev * GROUP + j)

                def flush():
                    while len(emitted) < GROUP:
                        cb()
                return cb, flush

            # software-pipelined emission: the PE queue at step t holds
            # [transposes(t), dots(t-1)], so while dots(t-1) waits for the
            # xt copy of tile t-1 (on DVE/ACT), the PE is already streaming
            # tile t's transposes -- the PE never head-of-line blocks on a
            # cross-engine copy.  Finals for group g-1 are interleaved one
            # group behind (tail_cb), and the recurrence for group g is
            # emitted as soon as its last u copy is queued.
            tail_cb = None
            flush = None
            for t in range(N_TILES + 1):
                if t < N_TILES:
                    emit_transposes(t)
                if t >= 1:
                    emit_dots(t - 1)
                    if tail_cb is not None:
                        tail_cb()
                    g, j = divmod(t - 1, GROUP)
                    if j == GROUP - 1:
                        if flush is not None:
                            flush()
                        emit_rec(g)
                        tail_cb, flush = make_tail_cb(g)
            flush()

    nc.compile()
    return nc


def _host_prep(weight_w, weight_b):
    """Host-side constants: wt_pack [128, 32] bf16, e_wide [128, 16] f32,
    ident [128, 128] bf16."""
    w64 = weight_w.astype(np.float64)
    b64 = weight_b.astype(np.float64)
    d = np.zeros((N_LAYER + 1, N_FEAT), dtype=np.float64)
    for i in range(N_LAYER):
        d[i + 1] = d[i] + b64[i]
    e = np.array([np.dot(d[i], w64[i]) for i in range(N_LAYER)],
                 dtype=np.float64)

    wbf = weight_w.astype(BF16)                       # [4, 1024]
    wt_pack = np.ascontiguousarray(
        wbf.T.reshape(N_BLK, P, N_LAYER).transpose(1, 0, 2).reshape(
            P, N_BLK * N_LAYER))                      # [p, f*4+i] = w[i, 128f+p]
    e_wide = np.ascontiguousarray(
        np.broadcast_to(
            np.repeat(e.astype(np.float32), GROUP)[None, :],
            (P, N_LAYER * GROUP)))
    ident = np.eye(P, dtype=BF16)
    return wt_pack, e_wide, ident


def _get_nc():
    if "nc" not in _CACHE:
        _CACHE["nc"] = _build_nc()
    return _CACHE["nc"]


def run(x, weight_w, weight_b, trace=False):
    """Run on 8 cores; returns (out_full, BassKernelResults)."""
    from concourse.bass_utils import run_bass_kernel_spmd

    x = np.ascontiguousarray(np.asarray(x, dtype=np.float32))
    weight_w = np.asarray(weight_w, dtype=np.float32)
    weight_b = np.asarray(weight_b, dtype=np.float32)
    assert x.shape == (B_FULL, N_FEAT)

    # partition-major device layout: [core][p, t, f] so DMA descriptors are
    # >=4KB contiguous per partition
    xb = np.ascontiguousarray(
        x.astype(BF16).reshape(N_CORES, N_TILES, P, N_FEAT)
        .transpose(0, 2, 1, 3).reshape(N_CORES, P, N_TILES * N_FEAT))
    wt_pack, e_wide, ident = _host_prep(weight_w, weight_b)

    nc = _get_nc()
    in_maps = [
        {
            "xb": xb[c],
            "wt_pack": wt_pack,
            "e_wide": e_wide,
            "ident": ident,
        }
        for c in range(N_CORES)
    ]
    res = run_bass_kernel_spmd(nc, in_maps, list(range(N_CORES)), trace=trace)
    out = np.concatenate(
        [res.results[c]["out"].reshape(P, N_TILES, N_FEAT).transpose(1, 0, 2)
         .reshape(B_LOCAL, N_FEAT).astype(np.float32)
         for c in range(N_CORES)],
        axis=0)
    return out, res


def kernel(x, weight_w, weight_b):
    out, _ = run(x, weight_w, weight_b, trace=False)
    return out
